# revision 39
# baseline (speedup 1.0000x reference)
"""Trainium2 Bass kernel for nn_DenseAttnProcessor (sparse_attention).

Cross-attention block: q = hs@Wq, k/v = ehs@{Wk,Wv}, per-head softmax((q k^T)/8
+ col_bias) @ v, @Wo + bo + residual.  B=8 batches -> data-parallel, one batch
per NeuronCore (no collectives).

Per-core dataflow (everything lives in "transposed" orientation so that every
matmul contraction has its operand already partition-major; softmax runs on
scoresT [T, q] with the per-head denominator handled by a ones-column matmul, a
reciprocal, and a K=1 broadcast matmul):

  stage A (once):  ehsT (host-pretransposed, bf16) -> k,v [77,1024] via matmul
                   -> kT via PE transpose -> M_h = v_h @ Wo_h [77,1024];
                   M rows DMA-packed into a [16*77+1, 1024] stack; the +bo
                   term rides as an extra stack row paired with an all-ones
                   probs row.
  stage B (8 chunks of 512 q rows):
                   hs chunk f32 -> bf16 cast -> PE-transpose -> hsT [C, q]
                   qT = Wq^T@hsT (psum accum over C) [inner, q]
                   per head: scoresT [77,512] = kT_h^T qT_h; z = Exp(scoresT)
                   * exp(col_bias)^T (host-precomputed multiplicative mask,
                   exact "set-column" suppression semantics, rows without
                   suppression are exactly 1.0); D = ones^T z; Dinv via fast
                   DVE reciprocal; DinvB via K=1 broadcast matmul; probsT =
                   z * DinvB, DMA-packed into the [16*77+1, 512] stack;
                   out[q,C] = sum_kt probsT_kt^T @ M_kt (10 K=128 matmuls)
                   + residual (f32) -> DMA out.  Chunks are software-
                   pipelined: softmax(ci-1) is emitted interleaved with
                   qT(ci) so the PE stream stays dense (HAM stays warm).

Inputs are the full unsharded arrays as produced by setup_inputs(); host side
only shards/casts/transposes small tensors and computes the tiny [2,77]/[2,4096]
suppression vectors.
"""

import sys

for _p in ("/opt/trn_rl_repo",):
    if _p not in sys.path:
        sys.path.insert(0, _p)

import numpy as np
import ml_dtypes

import concourse.mybir as mybir
import concourse.tile as tile
from concourse import bacc
from concourse.bass import ds
from concourse.masks import make_identity

F32 = mybir.dt.float32
BF16 = mybir.dt.bfloat16
AF = mybir.ActivationFunctionType

B, HW, C, CT, T, H, D = 8, 4096, 1024, 2048, 77, 16, 64
SUPPRESS = 20.0
RT = H * T + 1                # 1233 stacked rows (16*77 head rows + bo row)
NKT = (RT + 127) // 128       # 10 K-tiles for the AV matmul
NQ = 512                      # q rows per chunk
NCHUNK = HW // NQ             # 8
BO_TILE, BO_PART = (H * T) // 128, (H * T) % 128   # bo/ones row: tile 9, p 80


def _pack_pieces(h):
    """DMA pieces for packing head h's 77 rows at stacked row 77*h, split at
    128-row tile boundaries.  Returns list of (tile_idx, part_base, src_start,
    nrows).  (DMA writes have no partition-alignment restrictions.)"""
    g = T * h
    pieces = []
    pos = 0
    while pos < T:
        gg = g + pos
        ti, d = gg // 128, gg % 128
        n = min(T - pos, 128 - d)
        pieces.append((ti, d, pos, n))
        pos += n
    return pieces


def build_nc():
    nc = bacc.Bacc("TRN2", target_bir_lowering=False, debug=False)

    hs = nc.dram_tensor("hs", [HW, C], F32, kind="ExternalInput")
    ehsT = nc.dram_tensor("ehsT", [CT, T], BF16, kind="ExternalInput")
    wq = nc.dram_tensor("wq", [C, C], BF16, kind="ExternalInput")
    wk = nc.dram_tensor("wk", [CT, C], BF16, kind="ExternalInput")
    wv = nc.dram_tensor("wv", [CT, C], BF16, kind="ExternalInput")
    wo = nc.dram_tensor("wo", [C, C], BF16, kind="ExternalInput")
    euabt = nc.dram_tensor("euabt", [T, HW], BF16, kind="ExternalInput")
    bo = nc.dram_tensor("bo", [1, C], BF16, kind="ExternalInput")
    out = nc.dram_tensor("out", [HW, C], F32, kind="ExternalOutput")

    with tile.TileContext(nc) as tc:
        with (
            tc.tile_pool(name="const", bufs=1) as const,
            tc.tile_pool(name="persist", bufs=1) as persist,
        ):
            ident = const.tile([128, 128], BF16)
            make_identity(nc, ident)
            ones_col = const.tile([T, 1], BF16)
            nc.any.memset(ones_col, 1.0)
            ones_row = const.tile([1, T], BF16)
            nc.any.memset(ones_row, 1.0)
            ones_q = const.tile([1, NQ], BF16)
            nc.any.memset(ones_q, 1.0)
            eu_sb = const.tile([T, HW], BF16)
            nc.sync.dma_start(eu_sb, euabt[:, :])

            # persistent stacks
            kT_sb = persist.tile([128, C // 128, T], BF16)        # [inner, t]
            m_tiles = [persist.tile([128, C], BF16, name=f"m{i}") for i in range(NKT)]
            prob_bufs = [
                [persist.tile([128, NQ], BF16, name=f"pb{b}_{i}") for i in range(NKT)]
                for b in range(2)
            ]
            wq_tiles = [persist.tile([128, C], BF16, name=f"wqt{i}") for i in range(C // 128)]
            for i in range(C // 128):
                nc.sync.dma_start(wq_tiles[i], wq[ds(128 * i, 128), :])

            # only the last stack tile has rows past the packed head rows;
            # zero it so the AV matmuls see zeros there, then land bo/ones.
            nc.any.memset(m_tiles[BO_TILE], 0.0)
            for bset in prob_bufs:
                nc.any.memset(bset[BO_TILE], 0.0)
                nc.sync.dma_start(
                    bset[BO_TILE][BO_PART : BO_PART + 1, :], ones_q
                )
            nc.sync.dma_start(m_tiles[BO_TILE][BO_PART : BO_PART + 1, :], bo[:, :])

            # ---------------- stage A: k, v, kT, M ----------------
            with (
                tc.tile_pool(name="sa_sb", bufs=3) as sa_sb,
                tc.tile_pool(name="sa_w", bufs=3) as sa_w,
                tc.tile_pool(name="sa_ps", bufs=2, space="PSUM") as sa_ps,
            ):
                ehsT_sb = sa_sb.tile([128, CT // 128, T], BF16, bufs=1)
                for j in range(CT // 128):
                    nc.sync.dma_start(ehsT_sb[:, j, :], ehsT[ds(128 * j, 128), :])

                kv_sb = {}
                for name, wten in (("k", wk), ("v", wv)):
                    kv_ps = sa_ps.tile([T, C], F32, tag="kvps", bufs=1)
                    for j in range(CT // 128):
                        wt = sa_w.tile([128, C], BF16, tag="wkv")
                        nc.sync.dma_start(wt, wten[ds(128 * j, 128), :])
                        for nh in range(2):
                            nc.tensor.matmul(
                                kv_ps[:, ds(512 * nh, 512)],
                                ehsT_sb[:, j, :],
                                wt[:, ds(512 * nh, 512)],
                                start=(j == 0),
                                stop=(j == CT // 128 - 1),
                            )
                    kvs = sa_sb.tile([T, C], BF16, tag=f"{name}sb", bufs=1)
                    nc.any.tensor_copy(kvs, kv_ps)
                    kv_sb[name] = kvs

                # kT / vT via PE transpose of 128-column slices
                vT_sb = sa_sb.tile([128, C // 128, T], BF16, bufs=1)
                for src, dst in ((kv_sb["k"], kT_sb), (kv_sb["v"], vT_sb)):
                    for i in range(C // 128):
                        tp = sa_ps.tile([128, T], BF16, tag="tpa")
                        nc.tensor.transpose(tp, src[:, ds(128 * i, 128)], ident[:T, :T])
                        nc.any.tensor_copy(dst[:, i, :], tp)

                # M_h = v_h @ Wo_h, packed at stacked row 96h (+ bo at row 95)
                wot = None
                for h in range(H):
                    i, po = h // 2, (h % 2) * 64
                    if h % 2 == 0:
                        wot = sa_w.tile([128, C], BF16, tag="wot")
                        nc.sync.dma_start(wot, wo[ds(128 * i, 128), :])
                    m_ps = sa_ps.tile([T, C], F32, tag="mps")
                    for nh in range(2):
                        nc.tensor.matmul(
                            m_ps[:, ds(512 * nh, 512)],
                            vT_sb[ds(po, 64), i, :],
                            wot[ds(po, 64), ds(512 * nh, 512)],
                            start=True,
                            stop=True,
                        )
                    m_stg = sa_sb.tile([T, C], BF16, tag="mstg")
                    nc.any.tensor_copy(m_stg, m_ps)
                    for (ti, pb, s0, nr) in _pack_pieces(h):
                        nc.gpsimd.dma_start(
                            m_tiles[ti][ds(pb, nr), :], m_stg[ds(s0, nr), :]
                        )

            # ---------------- stage B: software-pipelined q chunks ----------------
            # Engine streams execute in emission order, so softmax(ci-1) head
            # chains are interleaved with qT(ci) matmul groups at build time:
            # the PE stream then always has dense matmul work queued and the
            # HAM clock gate stays open.
            with (
                tc.tile_pool(name="hsp", bufs=2) as hsp,
                tc.tile_pool(name="work", bufs=2) as work,
                tc.tile_pool(name="soft", bufs=4) as soft,
                tc.tile_pool(name="ops", bufs=2, space="PSUM") as ops,
            ):
                st = {}

                def load(ci):
                    q0 = NQ * ci
                    hs_f = hsp.tile([128, NQ // 128, C], F32, tag="hsf")
                    for qj in range(NQ // 128):
                        nc.sync.dma_start(
                            hs_f[:, qj, :], hs[ds(q0 + 128 * qj, 128), :]
                        )
                    hs_bf = work.tile([128, NQ // 128, C], BF16, tag="hsbf")
                    for qj in range(NQ // 128):
                        nc.scalar.copy(hs_bf[:, qj, :], hs_f[:, qj, :])
                    hsT = work.tile([128, C // 128, NQ], BF16, tag="hsT")
                    for cj in range(C // 128):
                        for qj in range(NQ // 128):
                            tp = ops.tile([128, 128], BF16, tag="pemm", bufs=2)
                            nc.tensor.transpose(
                                tp, hs_bf[:, qj, ds(128 * cj, 128)], ident
                            )
                            nc.any.tensor_copy(hsT[:, cj, ds(128 * qj, 128)], tp)
                    qT = work.tile([128, C // 128, NQ], BF16, tag="qT")
                    st[ci] = dict(hs_f=hs_f, hsT=hsT, qT=qT)

                def qt_group(ci, ij):
                    hsT, qT = st[ci]["hsT"], st[ci]["qT"]
                    q_ps = ops.tile([128, NQ], F32, tag="pemm", bufs=2)
                    for cj in range(C // 128):
                        nc.tensor.matmul(
                            q_ps,
                            wq_tiles[cj][:, ds(128 * ij, 128)],
                            hsT[:, cj, :],
                            start=(cj == 0),
                            stop=(cj == C // 128 - 1),
                        )
                    nc.any.tensor_copy(qT[:, ij, :], q_ps)

                def sm_head1(ci, h):
                    q0 = NQ * ci
                    qT = st[ci]["qT"]
                    i, po = h // 2, (h % 2) * 64
                    sT_ps = ops.tile([T, NQ], F32, tag="sT", bufs=2)
                    nc.tensor.matmul(
                        sT_ps,
                        kT_sb[ds(po, 64), i, :],
                        qT[ds(po, 64), i, :],
                        start=True,
                        stop=True,
                    )
                    expT = soft.tile([T, NQ], BF16, tag="expT", bufs=4)
                    nc.scalar.activation(expT, sT_ps, AF.Exp)
                    # multiplicative suppression mask exp(col_bias^T), host-
                    # precomputed; rows without suppression are exactly 1.0
                    z = soft.tile([T, NQ], BF16, tag="z", bufs=16, name=f"z{h}")
                    nc.vector.tensor_mul(z, expT, eu_sb[:, ds(q0, NQ)])
                    st[ci].setdefault("z", {})[h] = z

                def sm_head2(ci, h):
                    prob = prob_bufs[ci % 2]
                    z = st[ci]["z"][h]
                    d_ps = ops.tile([1, NQ], F32, tag="dps", bufs=1)
                    nc.tensor.matmul(d_ps, ones_col, z, start=True, stop=True)
                    dinv = soft.tile([1, NQ], F32, tag="dinv", bufs=2)
                    nc.vector.reciprocal_approx_fast(dinv, d_ps)
                    dinv_bf = soft.tile([1, NQ], BF16, tag="dinvbf", bufs=2)
                    nc.scalar.copy(dinv_bf, dinv)
                    db_ps = ops.tile([T, NQ], F32, tag="db", bufs=1)
                    nc.tensor.matmul(db_ps, ones_row, dinv_bf, start=True, stop=True)
                    p_stg = soft.tile([T, NQ], BF16, tag="pstg", bufs=4)
                    nc.vector.tensor_mul(p_stg, z, db_ps)
                    for (ti, pb, s0, nr) in _pack_pieces(h):
                        nc.sync.dma_start(
                            prob[ti][ds(pb, nr), :], p_stg[ds(s0, nr), :]
                        )

                def av(ci):
                    q0 = NQ * ci
                    prob = prob_bufs[ci % 2]
                    hs_f = st[ci]["hs_f"]
                    for qj in range(NQ // 128):
                        for nh in range(2):
                            o_ps = ops.tile([128, 512], F32, tag="ops", bufs=2)
                            for kt in range(NKT):
                                nc.tensor.matmul(
                                    o_ps,
                                    prob[kt][:, ds(128 * qj, 128)],
                                    m_tiles[kt][:, ds(512 * nh, 512)],
                                    start=(kt == 0),
                                    stop=(kt == NKT - 1),
                                )
                            o_sb = work.tile([128, 512], F32, tag="osb", bufs=3)
                            nc.vector.tensor_add(
                                o_sb, o_ps, hs_f[:, qj, ds(512 * nh, 512)]
                            )
                            nc.sync.dma_start(
                                out[ds(q0 + 128 * qj, 128), ds(512 * nh, 512)],
                                o_sb,
                            )

                load(0)
                for ij in range(C // 128):
                    qt_group(0, ij)
                for ci in range(1, NCHUNK + 1):
                    if ci < NCHUNK:
                        load(ci)
                    for h in range(H):
                        sm_head1(ci - 1, h)
                        if ci < NCHUNK and h % 2 == 0:
                            qt_group(ci, h // 2)
                    for h in range(H):
                        sm_head2(ci - 1, h)
                    av(ci - 1)

    nc.compile()
    return nc


_NC_CACHE = {}


def get_nc():
    if "nc" not in _NC_CACHE:
        _NC_CACHE["nc"] = build_nc()
    return _NC_CACHE["nc"]


def _bf16(x):
    return np.asarray(x, dtype=ml_dtypes.bfloat16)


def make_in_maps(inputs):
    hs = np.ascontiguousarray(np.asarray(inputs["hidden_states"], dtype=np.float32))
    ehs = np.asarray(inputs["encoder_hidden_states"], dtype=np.float32)
    mask_A = np.asarray(inputs["mask_A"], dtype=np.float32)
    mask_B = np.asarray(inputs["mask_B"], dtype=np.float32)
    Wq = np.asarray(inputs["Wq"], dtype=np.float32)
    Wk = np.asarray(inputs["Wk"], dtype=np.float32)
    Wv = np.asarray(inputs["Wv"], dtype=np.float32)
    Wo = np.asarray(inputs["Wo"], dtype=np.float32)
    bo = np.asarray(inputs["bo"], dtype=np.float32)
    idxA = np.asarray(inputs["token_indices_A"]).astype(np.int64) % T
    idxB = np.asarray(inputs["token_indices_B"]).astype(np.int64) % T

    # suppression as a multiplicative mask: exp(col_bias)^T [77, HW].
    # col_bias "set" semantics: B overwrites A; rows not in A|B are exactly 1.
    col_bias = np.zeros((HW, T), np.float32)
    col_bias[:, idxA] = (-SUPPRESS * (1.0 - mask_A))[:, None]
    col_bias[:, idxB] = (-SUPPRESS * (1.0 - mask_B))[:, None]
    euabt = np.exp(col_bias.T)

    scale = 1.0 / np.sqrt(D)
    wq_bf = _bf16(Wq * scale)
    wk_bf, wv_bf, wo_bf = _bf16(Wk), _bf16(Wv), _bf16(Wo)
    euabt_bf = _bf16(euabt)
    bo_bf = _bf16(bo[None, :])

    in_maps = []
    for b in range(B):
        in_maps.append(
            {
                "hs": hs[b],
                "ehsT": _bf16(ehs[b].T.copy()),
                "wq": wq_bf,
                "wk": wk_bf,
                "wv": wv_bf,
                "wo": wo_bf,
                "euabt": euabt_bf,
                "bo": bo_bf,
            }
        )
    return in_maps


def kernel(**inputs) -> np.ndarray:
    from concourse.bass_utils import run_bass_kernel_spmd

    nc = get_nc()
    in_maps = make_in_maps(inputs)
    res = run_bass_kernel_spmd(nc, in_maps, core_ids=list(range(B)))
    return np.stack([res.results[b]["out"] for b in range(B)]).astype(np.float32)


# revision 40
# speedup vs baseline: 1.0169x; 1.0169x over previous
"""Trainium2 Bass kernel for nn_DenseAttnProcessor (sparse_attention).

Cross-attention block: q = hs@Wq, k/v = ehs@{Wk,Wv}, per-head softmax((q k^T)/8
+ col_bias) @ v, @Wo + bo + residual.  B=8 batches -> data-parallel, one batch
per NeuronCore (no collectives).

Per-core dataflow (everything lives in "transposed" orientation so that every
matmul contraction has its operand already partition-major; softmax runs on
scoresT [T, q] with the per-head denominator handled by a ones-column matmul, a
reciprocal, and a K=1 broadcast matmul):

  stage A (once):  ehsT (host-pretransposed, bf16) -> k,v [77,1024] via matmul
                   -> kT via PE transpose -> M_h = v_h @ Wo_h [77,1024];
                   M rows DMA-packed into a [16*77+1, 1024] stack; the +bo
                   term rides as an extra stack row paired with an all-ones
                   probs row.
  stage B (8 chunks of 512 q rows):
                   hs chunk f32 -> bf16 cast -> PE-transpose -> hsT [C, q]
                   qT = Wq^T@hsT (psum accum over C) [inner, q]
                   per head: scoresT [77,512] = kT_h^T qT_h; z = Exp(scoresT)
                   * exp(col_bias)^T (host-precomputed multiplicative mask,
                   exact "set-column" suppression semantics, rows without
                   suppression are exactly 1.0); D = ones^T z; Dinv via fast
                   DVE reciprocal; DinvB via K=1 broadcast matmul; probsT =
                   z * DinvB, DMA-packed into the [16*77+1, 512] stack;
                   out[q,C] = sum_kt probsT_kt^T @ M_kt (10 K=128 matmuls)
                   + residual (f32) -> DMA out.  Chunks are software-
                   pipelined: softmax(ci-1) is emitted interleaved with
                   qT(ci) so the PE stream stays dense (HAM stays warm).

Inputs are the full unsharded arrays as produced by setup_inputs(); host side
only shards/casts/transposes small tensors and computes the tiny [2,77]/[2,4096]
suppression vectors.
"""

import sys

for _p in ("/opt/trn_rl_repo",):
    if _p not in sys.path:
        sys.path.insert(0, _p)

import numpy as np
import ml_dtypes

import concourse.mybir as mybir
import concourse.tile as tile
from concourse import bacc
from concourse.bass import ds
from concourse.masks import make_identity

F32 = mybir.dt.float32
BF16 = mybir.dt.bfloat16
AF = mybir.ActivationFunctionType

B, HW, C, CT, T, H, D = 8, 4096, 1024, 2048, 77, 16, 64
SUPPRESS = 20.0
RT = H * T + 1                # 1233 stacked rows (16*77 head rows + bo row)
NKT = (RT + 127) // 128       # 10 K-tiles for the AV matmul
NQ = 512                      # q rows per chunk
NCHUNK = HW // NQ             # 8
BO_TILE, BO_PART = (H * T) // 128, (H * T) % 128   # bo/ones row: tile 9, p 80


def _pack_pieces(h):
    """DMA pieces for packing head h's 77 rows at stacked row 77*h, split at
    128-row tile boundaries.  Returns list of (tile_idx, part_base, src_start,
    nrows).  (DMA writes have no partition-alignment restrictions.)"""
    g = T * h
    pieces = []
    pos = 0
    while pos < T:
        gg = g + pos
        ti, d = gg // 128, gg % 128
        n = min(T - pos, 128 - d)
        pieces.append((ti, d, pos, n))
        pos += n
    return pieces


def build_nc():
    nc = bacc.Bacc("TRN2", target_bir_lowering=False, debug=False)

    hs = nc.dram_tensor("hs", [HW, C], F32, kind="ExternalInput")
    ehsT = nc.dram_tensor("ehsT", [CT, T], BF16, kind="ExternalInput")
    wq = nc.dram_tensor("wq", [C, C], BF16, kind="ExternalInput")
    wk = nc.dram_tensor("wk", [CT, C], BF16, kind="ExternalInput")
    wv = nc.dram_tensor("wv", [CT, C], BF16, kind="ExternalInput")
    wo = nc.dram_tensor("wo", [C, C], BF16, kind="ExternalInput")
    euabt = nc.dram_tensor("euabt", [T, HW], BF16, kind="ExternalInput")
    bo = nc.dram_tensor("bo", [1, C], BF16, kind="ExternalInput")
    out = nc.dram_tensor("out", [HW, C], F32, kind="ExternalOutput")

    with tile.TileContext(nc) as tc:
        with (
            tc.tile_pool(name="const", bufs=1) as const,
            tc.tile_pool(name="persist", bufs=1) as persist,
        ):
            ident = const.tile([128, 128], BF16)
            make_identity(nc, ident)
            ones_col = const.tile([T, 1], BF16)
            nc.any.memset(ones_col, 1.0)
            ones_row = const.tile([1, T], BF16)
            nc.any.memset(ones_row, 1.0)
            ones_q = const.tile([1, NQ], BF16)
            nc.any.memset(ones_q, 1.0)
            eu_sb = const.tile([T, HW], BF16)
            nc.sync.dma_start(eu_sb, euabt[:, :])

            # persistent stacks
            kT_sb = persist.tile([128, C // 128, T], BF16)        # [inner, t]
            m_tiles = [persist.tile([128, C], BF16, name=f"m{i}") for i in range(NKT)]
            prob_bufs = [
                [persist.tile([128, NQ], BF16, name=f"pb{b}_{i}") for i in range(NKT)]
                for b in range(2)
            ]
            wq_tiles = [persist.tile([128, C], BF16, name=f"wqt{i}") for i in range(C // 128)]
            for i in range(C // 128):
                nc.sync.dma_start(wq_tiles[i], wq[ds(128 * i, 128), :])

            # only the last stack tile has rows past the packed head rows;
            # zero it so the AV matmuls see zeros there, then land bo/ones.
            nc.any.memset(m_tiles[BO_TILE], 0.0)
            for bset in prob_bufs:
                nc.any.memset(bset[BO_TILE], 0.0)
                nc.sync.dma_start(
                    bset[BO_TILE][BO_PART : BO_PART + 1, :], ones_q
                )
            nc.sync.dma_start(m_tiles[BO_TILE][BO_PART : BO_PART + 1, :], bo[:, :])

            # ---------------- stage A: k, v, kT, M ----------------
            with (
                tc.tile_pool(name="sa_sb", bufs=3) as sa_sb,
                tc.tile_pool(name="sa_w", bufs=3) as sa_w,
                tc.tile_pool(name="sa_ps", bufs=2, space="PSUM") as sa_ps,
            ):
                ehsT_sb = sa_sb.tile([128, CT // 128, T], BF16, bufs=1)
                for j in range(CT // 128):
                    nc.sync.dma_start(ehsT_sb[:, j, :], ehsT[ds(128 * j, 128), :])

                kv_sb = {}
                for name, wten in (("k", wk), ("v", wv)):
                    kv_ps = sa_ps.tile([T, C], F32, tag="kvps", bufs=1)
                    for j in range(CT // 128):
                        wt = sa_w.tile([128, C], BF16, tag="wkv")
                        nc.sync.dma_start(wt, wten[ds(128 * j, 128), :])
                        for nh in range(2):
                            nc.tensor.matmul(
                                kv_ps[:, ds(512 * nh, 512)],
                                ehsT_sb[:, j, :],
                                wt[:, ds(512 * nh, 512)],
                                start=(j == 0),
                                stop=(j == CT // 128 - 1),
                            )
                    kvs = sa_sb.tile([T, C], BF16, tag=f"{name}sb", bufs=1)
                    nc.any.tensor_copy(kvs, kv_ps)
                    kv_sb[name] = kvs

                # kT / vT via PE transpose of 128-column slices
                vT_sb = sa_sb.tile([128, C // 128, T], BF16, bufs=1)
                for src, dst in ((kv_sb["k"], kT_sb), (kv_sb["v"], vT_sb)):
                    for i in range(C // 128):
                        tp = sa_ps.tile([128, T], BF16, tag="tpa")
                        nc.tensor.transpose(tp, src[:, ds(128 * i, 128)], ident[:T, :T])
                        nc.any.tensor_copy(dst[:, i, :], tp)

                # M_h = v_h @ Wo_h, packed at stacked row 96h (+ bo at row 95)
                wot = None
                for h in range(H):
                    i, po = h // 2, (h % 2) * 64
                    if h % 2 == 0:
                        wot = sa_w.tile([128, C], BF16, tag="wot")
                        nc.sync.dma_start(wot, wo[ds(128 * i, 128), :])
                    m_ps = sa_ps.tile([T, C], F32, tag="mps")
                    for nh in range(2):
                        nc.tensor.matmul(
                            m_ps[:, ds(512 * nh, 512)],
                            vT_sb[ds(po, 64), i, :],
                            wot[ds(po, 64), ds(512 * nh, 512)],
                            start=True,
                            stop=True,
                        )
                    m_stg = sa_sb.tile([T, C], BF16, tag="mstg")
                    nc.any.tensor_copy(m_stg, m_ps)
                    for (ti, pb, s0, nr) in _pack_pieces(h):
                        nc.gpsimd.dma_start(
                            m_tiles[ti][ds(pb, nr), :], m_stg[ds(s0, nr), :]
                        )

            # ---------------- stage B: software-pipelined q chunks ----------------
            # Engine streams execute in emission order, so softmax(ci-1) head
            # chains are interleaved with qT(ci) matmul groups at build time:
            # the PE stream then always has dense matmul work queued and the
            # HAM clock gate stays open.
            with (
                tc.tile_pool(name="hsp", bufs=2) as hsp,
                tc.tile_pool(name="work", bufs=2) as work,
                tc.tile_pool(name="soft", bufs=4) as soft,
                tc.tile_pool(name="ops", bufs=2, space="PSUM") as ops,
            ):
                st = {}

                def load(ci):
                    q0 = NQ * ci
                    hs_f = hsp.tile([128, NQ // 128, C], F32, tag="hsf")
                    for qj in range(NQ // 128):
                        nc.sync.dma_start(
                            hs_f[:, qj, :], hs[ds(q0 + 128 * qj, 128), :]
                        )
                    hs_bf = work.tile([128, NQ // 128, C], BF16, tag="hsbf")
                    for qj in range(NQ // 128):
                        nc.scalar.copy(hs_bf[:, qj, :], hs_f[:, qj, :])
                    hsT = work.tile([128, C // 128, NQ], BF16, tag="hsT")
                    for qj in range(NQ // 128):
                        nc.sync.dma_start(
                            hsT[:, :, ds(128 * qj, 128)],
                            hs_bf[:, qj, :],
                            transpose=True,
                        )
                    qT = work.tile([128, C // 128, NQ], BF16, tag="qT")
                    st[ci] = dict(hs_f=hs_f, hsT=hsT, qT=qT)

                def qt_group(ci, ij):
                    hsT, qT = st[ci]["hsT"], st[ci]["qT"]
                    q_ps = ops.tile([128, NQ], F32, tag="qps", bufs=1)
                    for cj in range(C // 128):
                        nc.tensor.matmul(
                            q_ps,
                            wq_tiles[cj][:, ds(128 * ij, 128)],
                            hsT[:, cj, :],
                            start=(cj == 0),
                            stop=(cj == C // 128 - 1),
                        )
                    nc.any.tensor_copy(qT[:, ij, :], q_ps)

                def sm_head1(ci, h):
                    q0 = NQ * ci
                    qT = st[ci]["qT"]
                    i, po = h // 2, (h % 2) * 64
                    sT_ps = ops.tile([T, NQ], F32, tag="sT", bufs=2)
                    nc.tensor.matmul(
                        sT_ps,
                        kT_sb[ds(po, 64), i, :],
                        qT[ds(po, 64), i, :],
                        start=True,
                        stop=True,
                    )
                    expT = soft.tile([T, NQ], BF16, tag="expT", bufs=4)
                    nc.scalar.activation(expT, sT_ps, AF.Exp)
                    # multiplicative suppression mask exp(col_bias^T), host-
                    # precomputed; rows without suppression are exactly 1.0
                    z = soft.tile([T, NQ], BF16, tag="z", bufs=16, name=f"z{h}")
                    nc.vector.tensor_mul(z, expT, eu_sb[:, ds(q0, NQ)])
                    st[ci].setdefault("z", {})[h] = z

                def emit_d(ci, h):
                    d_ps = ops.tile([1, NQ], F32, tag="dps", bufs=2, name=f"dps{h}")
                    nc.tensor.matmul(d_ps, ones_col, st[ci]["z"][h], start=True, stop=True)
                    return d_ps

                def sm_head2(ci, h, d_ps, d_next):
                    prob = prob_bufs[ci % 2]
                    z = st[ci]["z"][h]
                    dinv = soft.tile([1, NQ], F32, tag="dinv", bufs=2)
                    nc.vector.reciprocal_approx_fast(dinv, d_ps)
                    dinv_bf = soft.tile([1, NQ], BF16, tag="dinvbf", bufs=2)
                    nc.scalar.copy(dinv_bf, dinv)
                    nxt = emit_d(ci, h + 1) if d_next else None
                    db_ps = ops.tile([T, NQ], F32, tag="db", bufs=1)
                    nc.tensor.matmul(db_ps, ones_row, dinv_bf, start=True, stop=True)
                    p_stg = soft.tile([T, NQ], BF16, tag="pstg", bufs=4)
                    nc.vector.tensor_mul(p_stg, z, db_ps)
                    for (ti, pb, s0, nr) in _pack_pieces(h):
                        nc.sync.dma_start(
                            prob[ti][ds(pb, nr), :], p_stg[ds(s0, nr), :]
                        )
                    return nxt

                def av(ci):
                    q0 = NQ * ci
                    prob = prob_bufs[ci % 2]
                    hs_f = st[ci]["hs_f"]
                    for qj in range(NQ // 128):
                        for nh in range(2):
                            o_ps = ops.tile([128, 512], F32, tag="ops", bufs=2)
                            for kt in range(NKT):
                                nc.tensor.matmul(
                                    o_ps,
                                    prob[kt][:, ds(128 * qj, 128)],
                                    m_tiles[kt][:, ds(512 * nh, 512)],
                                    start=(kt == 0),
                                    stop=(kt == NKT - 1),
                                )
                            o_sb = work.tile([128, 512], F32, tag="osb", bufs=3)
                            nc.vector.tensor_add(
                                o_sb, o_ps, hs_f[:, qj, ds(512 * nh, 512)]
                            )
                            nc.sync.dma_start(
                                out[ds(q0 + 128 * qj, 128), ds(512 * nh, 512)],
                                o_sb,
                            )

                load(0)
                for ij in range(C // 128):
                    qt_group(0, ij)
                for ci in range(1, NCHUNK + 1):
                    if ci < NCHUNK:
                        load(ci)
                    for h in range(H):
                        sm_head1(ci - 1, h)
                        if ci < NCHUNK and h % 2 == 0:
                            qt_group(ci, h // 2)
                    d_cur = emit_d(ci - 1, 0)
                    for h in range(H):
                        d_cur = sm_head2(ci - 1, h, d_cur, h + 1 < H)
                    av(ci - 1)

    nc.compile()
    return nc


_NC_CACHE = {}


def get_nc():
    if "nc" not in _NC_CACHE:
        _NC_CACHE["nc"] = build_nc()
    return _NC_CACHE["nc"]


def _bf16(x):
    return np.asarray(x, dtype=ml_dtypes.bfloat16)


def make_in_maps(inputs):
    hs = np.ascontiguousarray(np.asarray(inputs["hidden_states"], dtype=np.float32))
    ehs = np.asarray(inputs["encoder_hidden_states"], dtype=np.float32)
    mask_A = np.asarray(inputs["mask_A"], dtype=np.float32)
    mask_B = np.asarray(inputs["mask_B"], dtype=np.float32)
    Wq = np.asarray(inputs["Wq"], dtype=np.float32)
    Wk = np.asarray(inputs["Wk"], dtype=np.float32)
    Wv = np.asarray(inputs["Wv"], dtype=np.float32)
    Wo = np.asarray(inputs["Wo"], dtype=np.float32)
    bo = np.asarray(inputs["bo"], dtype=np.float32)
    idxA = np.asarray(inputs["token_indices_A"]).astype(np.int64) % T
    idxB = np.asarray(inputs["token_indices_B"]).astype(np.int64) % T

    # suppression as a multiplicative mask: exp(col_bias)^T [77, HW].
    # col_bias "set" semantics: B overwrites A; rows not in A|B are exactly 1.
    col_bias = np.zeros((HW, T), np.float32)
    col_bias[:, idxA] = (-SUPPRESS * (1.0 - mask_A))[:, None]
    col_bias[:, idxB] = (-SUPPRESS * (1.0 - mask_B))[:, None]
    euabt = np.exp(col_bias.T)

    scale = 1.0 / np.sqrt(D)
    wq_bf = _bf16(Wq * scale)
    wk_bf, wv_bf, wo_bf = _bf16(Wk), _bf16(Wv), _bf16(Wo)
    euabt_bf = _bf16(euabt)
    bo_bf = _bf16(bo[None, :])

    in_maps = []
    for b in range(B):
        in_maps.append(
            {
                "hs": hs[b],
                "ehsT": _bf16(ehs[b].T.copy()),
                "wq": wq_bf,
                "wk": wk_bf,
                "wv": wv_bf,
                "wo": wo_bf,
                "euabt": euabt_bf,
                "bo": bo_bf,
            }
        )
    return in_maps


def kernel(**inputs) -> np.ndarray:
    from concourse.bass_utils import run_bass_kernel_spmd

    nc = get_nc()
    in_maps = make_in_maps(inputs)
    res = run_bass_kernel_spmd(nc, in_maps, core_ids=list(range(B)))
    return np.stack([res.results[b]["out"] for b in range(B)]).astype(np.float32)
